# revision 12
# baseline (speedup 1.0000x reference)
"""Trainium2 Bass kernel for nn_CustomSTFT (STFT -> mag/phase -> iSTFT roundtrip).

Math: the reference computes real/imag via DFT-as-GEMM, converts to
(magnitude, phase) and immediately back to (rp, ip) = mag*(cos, sin)(phase).
Since cos(atan2(i, r)) = r/sqrt(r^2+i^2) exactly, the middle is the identity
up to a factor sqrt(1 + 1e-14/(r^2+i^2)) that is negligible (~1e-16 rel for
typical magnitudes ~O(10), and only reachable ~1e-8 abs in measure-zero
cases).  The whole module therefore collapses to a LINEAR map:

    wave = crop(overlap_add(frames @ A)),  A = Wfr.T @ Wbr - Wfi.T @ Wbi

Folding the overlap-add (hop 200, win 800 -> 4x overlap) into the matrix
gives a block-Toeplitz form on 200-sample blocks:

    out_block[g] = sum_{d=-3..3} u[g+d] @ C_d,   C_d = sum_j A_blk[j+d, j]

which is 2800 FLOPs/sample instead of ~6400 (and ~12800 for the reference's
4 separate GEMMs).  Two boundary blocks need small corrections (frames f=-1
and f=2401 do not exist); these are 6 extra tiny GEMMs.

Device kernel (SPMD over 8 cores, 4 batch rows each): x is laid out
transposed as [k=200 partitions (2 chunks of 128/72), block columns], so
the 7 Toeplitz shifts become column offsets into the same SBUF tile.
Matmuls run in float32r (full PE rate at N>=256) accumulating in fp32 PSUM.
"""

import os
import numpy as np

# ---------------- problem constants (hardcoded per contract) ----------------
B, T = 32, 480000
H = 200            # hop
NFFT = 800
PAD = 400
N_CORES = 8
BPC = B // N_CORES          # 4 batch rows per core
NBLK = (T + 2 * PAD) // H   # 2404 input blocks per batch (padded signal)
NCOL = NBLK + 2             # + zero border column on each side = 2406
G = T // H                  # 2400 output blocks per batch
GRP = 480                   # output columns per PSUM accumulation group
NGRP = G // GRP             # 5
KC = ((0, 128), (128, 72))  # contraction (k) chunks over the 200-dim
CC = ((0, 128), (128, 72))  # output-channel (c) chunks over the 200-dim

_MM_DTYPE = os.environ.get("STFT_MM_DTYPE", "float32r")

_CACHE = {}


# ---------------- host-side weight folding (fp64) ----------------
def _fold_weights(wfr, wfi, wbr, wbi):
    wfr = np.asarray(wfr, dtype=np.float64)
    wfi = np.asarray(wfi, dtype=np.float64)
    wbr = np.asarray(wbr, dtype=np.float64)
    wbi = np.asarray(wbi, dtype=np.float64)
    A = wfr.T @ wbr - wfi.T @ wbi  # [800, 800]
    Ab = A.reshape(4, H, 4, H)     # [r, k, j, c] blocks
    C = np.zeros((7, H, H))
    for d in range(-3, 4):
        for j in range(4):
            r = j + d
            if 0 <= r <= 3:
                C[d + 3] += Ab[r, :, j, :]
    # cm[k, (d+3)*H + c] = C[d, k, c]
    cm = np.ascontiguousarray(C.transpose(1, 0, 2).reshape(H, 7 * H))
    # edge corrections, NEGATED so the device just accumulates and adds.
    # lo (first out block, g=2):  -= sum_t u[t]      @ A_blk[1+t, 3]
    # hi (last out block, g=2401): -= sum_t u[2401+t] @ A_blk[t, 0]
    E = np.zeros((2, 3, H, H))
    for t in range(3):
        E[0, t] = -Ab[1 + t, :, 3, :]
        E[1, t] = -Ab[t, :, 0, :]
    # ce[k, (e*3+t)*H + c] = E[e, t, k, c]
    ce = np.ascontiguousarray(E.transpose(2, 0, 1, 3).reshape(H, 6 * H))
    return cm.astype(np.float32), ce.astype(np.float32)


# ---------------- bass program ----------------
def _build_nc():
    import concourse.bass as bass
    import concourse.mybir as mybir
    from concourse.tile import TileContext
    from concourse.tile_rust import add_dep_helper

    mmdt = getattr(mybir.dt, _MM_DTYPE)
    f32 = mybir.dt.float32

    nc = bass.Bass()
    xt_d = nc.declare_dram_parameter("xt", [H, BPC * NCOL], mmdt, False)
    cm_d = nc.declare_dram_parameter("cm", [H, 7 * H], mmdt, False)
    ce_d = nc.declare_dram_parameter("ce", [H, 6 * H], mmdt, False)
    eg_d = nc.declare_dram_parameter("eg", [H, 6 * BPC], mmdt, False)
    yt_d = nc.declare_dram_parameter("yt", [H, BPC * G], f32, True)

    with TileContext(nc) as tc:
        with (
            tc.tile_pool(name="wpool", bufs=1) as wpool,
            tc.tile_pool(name="xpool", bufs=1) as xpool,
            tc.tile_pool(name="opool0", bufs=4) as opool0,
            tc.tile_pool(name="opool1", bufs=4) as opool1,
            tc.tile_pool(name="epool", bufs=1) as epool,
            tc.tile_pool(name="pmain", bufs=6, space="PSUM") as pmain,
            tc.tile_pool(name="pedge", bufs=1, space="PSUM") as pedge,
            tc.tile_pool(name="pscr", bufs=1, space="PSUM") as pscr,
        ):
            opools = (opool0, opool1)
            cm_t, ce_t, xt_t, eg_t = {}, {}, {}, {}
            for kci, (k0, kn) in enumerate(KC):
                cm_t[kci] = wpool.tile([kn, 7 * H], mmdt, name=f"cm{kci}", tag=f"cm{kci}")
                nc.gpsimd.dma_start(out=cm_t[kci][:], in_=cm_d[k0:k0 + kn, :])
                ce_t[kci] = wpool.tile([kn, 6 * H], mmdt, name=f"ce{kci}", tag=f"ce{kci}")
                nc.gpsimd.dma_start(out=ce_t[kci][:], in_=ce_d[k0:k0 + kn, :])
                eg_t[kci] = epool.tile([kn, 6 * BPC], mmdt, name=f"eg{kci}", tag=f"eg{kci}")
                nc.gpsimd.dma_start(out=eg_t[kci][:], in_=eg_d[k0:k0 + kn, :])
                xt_t[kci] = xpool.tile([kn, BPC * NCOL], mmdt, name=f"xt{kci}", tag=f"xt{kci}")
            # per-batch input chunks, weights first, batches in consumption
            # order (SWDGE keeps them off the output HWDGE queues)
            for b in range(BPC):
                for kci, (k0, kn) in enumerate(KC):
                    nc.gpsimd.dma_start(
                        out=xt_t[kci][:, b * NCOL:(b + 1) * NCOL],
                        in_=xt_d[k0:k0 + kn, b * NCOL:(b + 1) * NCOL],
                    )

            # --- toll-gate matmuls -------------------------------------
            # walrus can fuse at most ONE sync-wait into a self-loading
            # fp32r matmul.  Real PE instructions advance Tile's observed
            # vector clock (sequencer NOPs do not), so absorb each input
            # DMA's semaphore with a tiny matmul touching just that tile;
            # later matmuls then need no fused DMA waits.  no_sync edges pin
            # every real matmul after its gate.
            scr = pscr.tile([2, 2], f32, name="scr", tag="scr")
            gates = []

            def gate(ap):
                ap = ap.bitcast(f32)
                mm = nc.tensor.matmul(scr[:], ap, ap, start=True, stop=True)
                if gates:
                    add_dep_helper(mm.ins, gates[-1].ins, sync=False,
                                   reason="order gates")
                gates.append(mm)

            def after_gate(mm):
                add_dep_helper(mm.ins, gates[-1].ins, sync=False,
                               reason="after toll gates")

            for kci in range(len(KC)):
                gate(cm_t[kci][0:32, 0:2])
                gate(ce_t[kci][0:32, 0:2])
                gate(eg_t[kci][0:32, 0:2])

            # edge-correction matmuls -> esb[cci][:, e*BPC + b]
            esb_t = {}
            for cci, (c0, cn) in enumerate(CC):
                esb_t[cci] = epool.tile([cn, 2 * BPC], f32, name=f"esb{cci}", tag=f"esb{cci}")
                for e in range(2):
                    pe_t = pedge.tile([cn, BPC], f32, name="pe_t", tag="pe_t")
                    idx = 0
                    for t in range(3):
                        for kci, (k0, kn) in enumerate(KC):
                            s = (e * 3 + t)
                            after_gate(nc.tensor.matmul(
                                pe_t[:],
                                ce_t[kci][:, s * H + c0: s * H + c0 + cn],
                                eg_t[kci][:, s * BPC:(s + 1) * BPC],
                                start=(idx == 0),
                                stop=(idx == 5),
                            ))
                            idx += 1
                    nc.vector.tensor_copy(
                        out=esb_t[cci][:, e * BPC:(e + 1) * BPC], in_=pe_t[:]
                    )
            # main block-Toeplitz GEMM
            for b in range(BPC):
                for kci in range(len(KC)):
                    gate(xt_t[kci][0:32, b * NCOL:b * NCOL + 2])
                ots = {}
                for cci, (c0, cn) in enumerate(CC):
                    ot = opools[cci].tile([cn, G], f32, name=f"ot{cci}", tag=f"ot{cci}")
                    ots[cci] = ot
                    for grp in range(NGRP):
                        o0 = grp * GRP
                        ps = pmain.tile([cn, GRP], f32, name="ps", tag="ps")
                        idx = 0
                        for d in range(-3, 4):
                            for kci, (k0, kn) in enumerate(KC):
                                after_gate(nc.tensor.matmul(
                                    ps[:],
                                    cm_t[kci][:, (d + 3) * H + c0:(d + 3) * H + c0 + cn],
                                    xt_t[kci][:, b * NCOL + o0 + 3 + d:
                                              b * NCOL + o0 + 3 + d + GRP],
                                    start=(idx == 0),
                                    stop=(idx == 13),
                                ))
                                idx += 1
                        nc.vector.tensor_copy(out=ot[:, o0:o0 + GRP], in_=ps[:])
                    # boundary-block corrections
                    nc.vector.tensor_add(
                        out=ot[:, 0:1], in0=ot[:, 0:1], in1=esb_t[cci][:, b:b + 1]
                    )
                    nc.vector.tensor_add(
                        out=ot[:, G - 1:G], in0=ot[:, G - 1:G],
                        in1=esb_t[cci][:, BPC + b:BPC + b + 1],
                    )
                for cci, (c0, cn) in enumerate(CC):
                    nc.sync.dma_start(
                        out=yt_d[c0:c0 + cn, b * G:(b + 1) * G], in_=ots[cci][:]
                    )
    return nc


def _legalize_waits(nc):
    """walrus fuses at most ONE sync-wait into most instructions (and the
    Tile kernel-tail drain gets one per outstanding proc).  Split extras
    into preceding single-wait NoOps on the same engine."""
    import concourse.mybir as mybir

    for f in nc.m.functions:
        for blk in f.blocks:
            new, changed = [], False
            for inst in blk.instructions:
                si = inst.sync_info
                if si is not None and si.on_wait and len(si.on_wait) > 1:
                    waits = list(si.on_wait)
                    for i, w in enumerate(waits[:-1]):
                        nop = mybir.InstNoOp(
                            name=f"{inst.name}-waitsplit{i}", ins=[], outs=[])
                        nop.engine = inst.engine
                        nop.sync_info = mybir.SyncInfo(on_wait=[w], on_update=[])
                        new.append(nop)
                    inst.sync_info = mybir.SyncInfo(
                        on_wait=[waits[-1]], on_update=list(si.on_update or []))
                    changed = True
                new.append(inst)
            if changed:
                blk.instructions = new


def _get_nc():
    if "nc" not in _CACHE:
        nc = _build_nc()
        _legalize_waits(nc)
        _CACHE["nc"] = nc
    return _CACHE["nc"]


# ---------------- host-side data layout ----------------
def _prep_x(x):
    """x [B, T] f32 -> per-core xt [H, BPC*NCOL] f32, transposed block layout
    with one zero border column per batch on each side; plus per-core edge
    input columns eg [H, 6*BPC] (lo: blocks 0..2, hi: blocks 2401..2403)."""
    xp = np.pad(np.asarray(x, dtype=np.float32), ((0, 0), (PAD, PAD)), mode="edge")
    blocks = xp.reshape(B, NBLK, H)
    xts, egs = [], []
    for c in range(N_CORES):
        cb = blocks[c * BPC:(c + 1) * BPC]          # [BPC, NBLK, H]
        xt = np.zeros((H, BPC, NCOL), dtype=np.float32)
        # xt[k, b, i] = xp[core_b, (i-1)*H + k]
        xt[:, :, 1:NCOL - 1] = cb.transpose(2, 0, 1)
        xts.append(np.ascontiguousarray(xt.reshape(H, BPC * NCOL)))
        eg = np.empty((H, 2, 3, BPC), dtype=np.float32)
        for t in range(3):
            eg[:, 0, t, :] = cb[:, t, :].T           # u[t]
            eg[:, 1, t, :] = cb[:, 2401 + t, :].T    # u[2401+t]
        egs.append(np.ascontiguousarray(eg.reshape(H, 6 * BPC)))
    return xts, egs


def _gather_y(results):
    out = np.empty((B, T), dtype=np.float32)
    for c in range(N_CORES):
        yt = results[c]["yt"].reshape(H, BPC, G)
        out[c * BPC:(c + 1) * BPC] = (
            yt.transpose(1, 2, 0).reshape(BPC, T)
        )
    return out


# ---------------- entry point ----------------
def kernel(x, w_fwd_real, w_fwd_imag, w_bwd_real, w_bwd_imag, **_):
    from concourse.bass_utils import run_bass_kernel_spmd

    cm, ce = _fold_weights(w_fwd_real, w_fwd_imag, w_bwd_real, w_bwd_imag)
    xts, egs = _prep_x(x)
    in_maps = [{"xt": xts[c], "cm": cm, "ce": ce, "eg": egs[c]}
               for c in range(N_CORES)]
    nc = _get_nc()
    res = run_bass_kernel_spmd(nc, in_maps, list(range(N_CORES)))
    return _gather_y(res.results)


# revision 13
# speedup vs baseline: 1.0547x; 1.0547x over previous
"""Trainium2 Bass kernel for nn_CustomSTFT (STFT -> mag/phase -> iSTFT roundtrip).

Math: the reference computes real/imag via DFT-as-GEMM, converts to
(magnitude, phase) and immediately back to (rp, ip) = mag*(cos, sin)(phase).
Since cos(atan2(i, r)) = r/sqrt(r^2+i^2) exactly, the middle is the identity
up to a factor sqrt(1 + 1e-14/(r^2+i^2)) that is negligible (~1e-16 rel for
typical magnitudes ~O(10), and only reachable ~1e-8 abs in measure-zero
cases).  The whole module therefore collapses to a LINEAR map:

    wave = crop(overlap_add(frames @ A)),  A = Wfr.T @ Wbr - Wfi.T @ Wbi

Folding the overlap-add (hop 200, win 800 -> 4x overlap) into the matrix
gives a block-Toeplitz form on 200-sample blocks:

    out_block[g] = sum_{d=-3..3} u[g+d] @ C_d,   C_d = sum_j A_blk[j+d, j]

which is 2800 FLOPs/sample instead of ~6400 (and ~12800 for the reference's
4 separate GEMMs).  Two boundary blocks need small corrections (frames f=-1
and f=2401 do not exist); these are 6 extra tiny GEMMs.

Device kernel (SPMD over 8 cores, 4 batch rows each): x is laid out
transposed as [k=200 partitions (2 chunks of 128/72), block columns], so
the 7 Toeplitz shifts become column offsets into the same SBUF tile.
Matmuls run in float32r (full PE rate at N>=256) accumulating in fp32 PSUM.
"""

import os
import numpy as np

# ---------------- problem constants (hardcoded per contract) ----------------
B, T = 32, 480000
H = 200            # hop
NFFT = 800
PAD = 400
N_CORES = 8
BPC = B // N_CORES          # 4 batch rows per core
NBLK = (T + 2 * PAD) // H   # 2404 input blocks per batch (padded signal)
NCOL = NBLK + 2             # + zero border column on each side = 2406
G = T // H                  # 2400 output blocks per batch
GRP = 480                   # output columns per PSUM accumulation group
NGRP = G // GRP             # 5
KC = ((0, 128), (128, 72))  # contraction (k) chunks over the 200-dim
CC = ((0, 128), (128, 72))  # output-channel (c) chunks over the 200-dim

_MM_DTYPE = os.environ.get("STFT_MM_DTYPE", "float32r")

_CACHE = {}


# ---------------- host-side weight folding (fp64) ----------------
def _fold_weights(wfr, wfi, wbr, wbi):
    wfr = np.asarray(wfr, dtype=np.float64)
    wfi = np.asarray(wfi, dtype=np.float64)
    wbr = np.asarray(wbr, dtype=np.float64)
    wbi = np.asarray(wbi, dtype=np.float64)
    A = wfr.T @ wbr - wfi.T @ wbi  # [800, 800]
    Ab = A.reshape(4, H, 4, H)     # [r, k, j, c] blocks
    C = np.zeros((7, H, H))
    for d in range(-3, 4):
        for j in range(4):
            r = j + d
            if 0 <= r <= 3:
                C[d + 3] += Ab[r, :, j, :]
    # cm[k, (d+3)*H + c] = C[d, k, c]
    cm = np.ascontiguousarray(C.transpose(1, 0, 2).reshape(H, 7 * H))
    # edge corrections, NEGATED so the device just accumulates and adds.
    # lo (first out block, g=2):  -= sum_t u[t]      @ A_blk[1+t, 3]
    # hi (last out block, g=2401): -= sum_t u[2401+t] @ A_blk[t, 0]
    E = np.zeros((2, 3, H, H))
    for t in range(3):
        E[0, t] = -Ab[1 + t, :, 3, :]
        E[1, t] = -Ab[t, :, 0, :]
    # ce[k, (e*3+t)*H + c] = E[e, t, k, c]
    ce = np.ascontiguousarray(E.transpose(2, 0, 1, 3).reshape(H, 6 * H))
    return cm.astype(np.float32), ce.astype(np.float32)


# ---------------- bass program ----------------
def _build_nc():
    import concourse.bass as bass
    import concourse.mybir as mybir
    from concourse.tile import TileContext

    mmdt = getattr(mybir.dt, _MM_DTYPE)
    f32 = mybir.dt.float32

    nc = bass.Bass()
    xt_d = nc.declare_dram_parameter("xt", [H, BPC * NCOL], mmdt, False)
    cm_d = nc.declare_dram_parameter("cm", [H, 7 * H], mmdt, False)
    ce_d = nc.declare_dram_parameter("ce", [H, 6 * H], mmdt, False)
    eg_d = nc.declare_dram_parameter("eg", [H, 6 * BPC], mmdt, False)
    yt_d = nc.declare_dram_parameter("yt", [H, BPC * G], f32, True)

    with TileContext(nc) as tc:
        with (
            tc.tile_pool(name="wpool", bufs=1) as wpool,
            tc.tile_pool(name="xpool", bufs=1) as xpool,
            tc.tile_pool(name="opool0", bufs=4) as opool0,
            tc.tile_pool(name="opool1", bufs=4) as opool1,
            tc.tile_pool(name="epool", bufs=1) as epool,
            tc.tile_pool(name="pmain", bufs=6, space="PSUM") as pmain,
            tc.tile_pool(name="pedge", bufs=2, space="PSUM") as pedge,
        ):
            opools = (opool0, opool1)
            cm_t, ce_t, xt_t, eg_t = {}, {}, {}, {}
            # weights + edge columns first (small, unblock PE quickly),
            # then x chunks in consumption order, all on the 8 HWDGE queues
            for kci, (k0, kn) in enumerate(KC):
                cm_t[kci] = wpool.tile([kn, 7 * H], mmdt, name=f"cm{kci}", tag=f"cm{kci}")
                nc.sync.dma_start(out=cm_t[kci][:], in_=cm_d[k0:k0 + kn, :])
                ce_t[kci] = wpool.tile([kn, 6 * H], mmdt, name=f"ce{kci}", tag=f"ce{kci}")
                nc.sync.dma_start(out=ce_t[kci][:], in_=ce_d[k0:k0 + kn, :])
                eg_t[kci] = epool.tile([kn, 6 * BPC], mmdt, name=f"eg{kci}", tag=f"eg{kci}")
                nc.sync.dma_start(out=eg_t[kci][:], in_=eg_d[k0:k0 + kn, :])
                xt_t[kci] = xpool.tile([kn, BPC * NCOL], mmdt, name=f"xt{kci}", tag=f"xt{kci}")
            for b in range(BPC):
                for kci, (k0, kn) in enumerate(KC):
                    nc.sync.dma_start(
                        out=xt_t[kci][:, b * NCOL:(b + 1) * NCOL],
                        in_=xt_d[k0:k0 + kn, b * NCOL:(b + 1) * NCOL],
                    )

            esb_t = {}
            for cci, (c0, cn) in enumerate(CC):
                esb_t[cci] = epool.tile([cn, 2 * BPC], f32, name=f"esb{cci}", tag=f"esb{cci}")

            def emit_edges():
                # edge-correction matmuls -> esb[cci][:, e*BPC + b]
                for cci, (c0, cn) in enumerate(CC):
                    for e in range(2):
                        pe_t = pedge.tile([cn, BPC], f32, name="pe_t", tag="pe_t")
                        idx = 0
                        for t in range(3):
                            for kci, (k0, kn) in enumerate(KC):
                                s = (e * 3 + t)
                                nc.tensor.matmul(
                                    pe_t[:],
                                    ce_t[kci][:, s * H + c0: s * H + c0 + cn],
                                    eg_t[kci][:, s * BPC:(s + 1) * BPC],
                                    start=(idx == 0),
                                    stop=(idx == 5),
                                )
                                idx += 1
                        nc.vector.tensor_copy(
                            out=esb_t[cci][:, e * BPC:(e + 1) * BPC], in_=pe_t[:]
                        )

            def emit_batch_main(b):
                ots = {}
                for cci, (c0, cn) in enumerate(CC):
                    ot = opools[cci].tile([cn, G], f32, name=f"ot{cci}", tag=f"ot{cci}")
                    ots[cci] = ot
                    for grp in range(NGRP):
                        o0 = grp * GRP
                        ps = pmain.tile([cn, GRP], f32, name="ps", tag="ps")
                        idx = 0
                        for d in range(-3, 4):
                            for kci, (k0, kn) in enumerate(KC):
                                nc.tensor.matmul(
                                    ps[:],
                                    cm_t[kci][:, (d + 3) * H + c0:(d + 3) * H + c0 + cn],
                                    xt_t[kci][:, b * NCOL + o0 + 3 + d:
                                              b * NCOL + o0 + 3 + d + GRP],
                                    start=(idx == 0),
                                    stop=(idx == 13),
                                )
                                idx += 1
                        nc.vector.tensor_copy(out=ot[:, o0:o0 + GRP], in_=ps[:])
                        if 0 < grp < NGRP - 1:
                            # interior groups stream out immediately
                            nc.sync.dma_start(
                                out=yt_d[c0:c0 + cn, b * G + o0:b * G + o0 + GRP],
                                in_=ot[:, o0:o0 + GRP],
                            )
                return ots

            def emit_batch_edges_and_out(b, ots):
                # boundary-block corrections, then first/last group out-DMAs
                for cci, (c0, cn) in enumerate(CC):
                    ot = ots[cci]
                    nc.vector.tensor_add(
                        out=ot[:, 0:1], in0=ot[:, 0:1], in1=esb_t[cci][:, b:b + 1]
                    )
                    nc.vector.tensor_add(
                        out=ot[:, G - 1:G], in0=ot[:, G - 1:G],
                        in1=esb_t[cci][:, BPC + b:BPC + b + 1],
                    )
                    for grp in (0, NGRP - 1):
                        o0 = grp * GRP
                        nc.sync.dma_start(
                            out=yt_d[c0:c0 + cn, b * G + o0:b * G + o0 + GRP],
                            in_=ot[:, o0:o0 + GRP],
                        )

            ots0 = emit_batch_main(0)
            emit_edges()
            emit_batch_edges_and_out(0, ots0)
            for b in range(1, BPC):
                ots = emit_batch_main(b)
                emit_batch_edges_and_out(b, ots)
    return nc


def _legalize_waits(nc):
    """walrus fuses at most ONE sync-wait into most instructions (and the
    Tile kernel-tail drain gets one per outstanding proc).  Split extras
    into preceding single-wait NoOps on the same engine."""
    import concourse.mybir as mybir

    for f in nc.m.functions:
        for blk in f.blocks:
            new, changed = [], False
            for inst in blk.instructions:
                si = inst.sync_info
                if si is not None and si.on_wait and len(si.on_wait) > 1:
                    waits = list(si.on_wait)
                    for i, w in enumerate(waits[:-1]):
                        nop = mybir.InstNoOp(
                            name=f"{inst.name}-waitsplit{i}", ins=[], outs=[])
                        nop.engine = inst.engine
                        nop.sync_info = mybir.SyncInfo(on_wait=[w], on_update=[])
                        new.append(nop)
                    inst.sync_info = mybir.SyncInfo(
                        on_wait=[waits[-1]], on_update=list(si.on_update or []))
                    changed = True
                new.append(inst)
            if changed:
                blk.instructions = new


def _get_nc():
    if "nc" not in _CACHE:
        nc = _build_nc()
        _legalize_waits(nc)
        _CACHE["nc"] = nc
    return _CACHE["nc"]


# ---------------- host-side data layout ----------------
def _prep_x(x):
    """x [B, T] f32 -> per-core xt [H, BPC*NCOL] f32, transposed block layout
    with one zero border column per batch on each side; plus per-core edge
    input columns eg [H, 6*BPC] (lo: blocks 0..2, hi: blocks 2401..2403)."""
    xp = np.pad(np.asarray(x, dtype=np.float32), ((0, 0), (PAD, PAD)), mode="edge")
    blocks = xp.reshape(B, NBLK, H)
    xts, egs = [], []
    for c in range(N_CORES):
        cb = blocks[c * BPC:(c + 1) * BPC]          # [BPC, NBLK, H]
        xt = np.zeros((H, BPC, NCOL), dtype=np.float32)
        # xt[k, b, i] = xp[core_b, (i-1)*H + k]
        xt[:, :, 1:NCOL - 1] = cb.transpose(2, 0, 1)
        xts.append(np.ascontiguousarray(xt.reshape(H, BPC * NCOL)))
        eg = np.empty((H, 2, 3, BPC), dtype=np.float32)
        for t in range(3):
            eg[:, 0, t, :] = cb[:, t, :].T           # u[t]
            eg[:, 1, t, :] = cb[:, 2401 + t, :].T    # u[2401+t]
        egs.append(np.ascontiguousarray(eg.reshape(H, 6 * BPC)))
    return xts, egs


def _gather_y(results):
    out = np.empty((B, T), dtype=np.float32)
    for c in range(N_CORES):
        yt = results[c]["yt"].reshape(H, BPC, G)
        out[c * BPC:(c + 1) * BPC] = (
            yt.transpose(1, 2, 0).reshape(BPC, T)
        )
    return out


# ---------------- entry point ----------------
def kernel(x, w_fwd_real, w_fwd_imag, w_bwd_real, w_bwd_imag, **_):
    from concourse.bass_utils import run_bass_kernel_spmd

    cm, ce = _fold_weights(w_fwd_real, w_fwd_imag, w_bwd_real, w_bwd_imag)
    xts, egs = _prep_x(x)
    in_maps = [{"xt": xts[c], "cm": cm, "ce": ce, "eg": egs[c]}
               for c in range(N_CORES)]
    nc = _get_nc()
    res = run_bass_kernel_spmd(nc, in_maps, list(range(N_CORES)))
    return _gather_y(res.results)


# revision 14
# speedup vs baseline: 1.0647x; 1.0095x over previous
"""Trainium2 Bass kernel for nn_CustomSTFT (STFT -> mag/phase -> iSTFT roundtrip).

Math: the reference computes real/imag via DFT-as-GEMM, converts to
(magnitude, phase) and immediately back to (rp, ip) = mag*(cos, sin)(phase).
Since cos(atan2(i, r)) = r/sqrt(r^2+i^2) exactly, the middle is the identity
up to a factor sqrt(1 + 1e-14/(r^2+i^2)) that is negligible (~1e-16 rel for
typical magnitudes ~O(10), and only reachable ~1e-8 abs in measure-zero
cases).  The whole module therefore collapses to a LINEAR map:

    wave = crop(overlap_add(frames @ A)),  A = Wfr.T @ Wbr - Wfi.T @ Wbi

Folding the overlap-add (hop 200, win 800 -> 4x overlap) into the matrix
gives a block-Toeplitz form on 200-sample blocks:

    out_block[g] = sum_{d=-3..3} u[g+d] @ C_d,   C_d = sum_j A_blk[j+d, j]

which is 2800 FLOPs/sample instead of ~6400 (and ~12800 for the reference's
4 separate GEMMs).  Two boundary blocks need small corrections (frames f=-1
and f=2401 do not exist); these are 6 extra tiny GEMMs.

Device kernel (SPMD over 8 cores, 4 batch rows each): x is laid out
transposed as [k=200 partitions (2 chunks of 128/72), block columns], so
the 7 Toeplitz shifts become column offsets into the same SBUF tile.
Matmuls run in float32r (full PE rate at N>=256) accumulating in fp32 PSUM.
"""

import os
import numpy as np

# ---------------- problem constants (hardcoded per contract) ----------------
B, T = 32, 480000
H = 200            # hop
NFFT = 800
PAD = 400
N_CORES = 8
BPC = B // N_CORES          # 4 batch rows per core
NBLK = (T + 2 * PAD) // H   # 2404 input blocks per batch (padded signal)
NCOL = NBLK + 2             # + zero border column on each side = 2406
G = T // H                  # 2400 output blocks per batch
GRP = 480                   # output columns per PSUM accumulation group
NGRP = G // GRP             # 5
KC = ((0, 128), (128, 72))  # contraction (k) chunks over the 200-dim
CC = ((0, 128), (128, 72))  # output-channel (c) chunks over the 200-dim

_MM_DTYPE = os.environ.get("STFT_MM_DTYPE", "float32r")

_CACHE = {}


# ---------------- host-side weight folding (fp64) ----------------
def _fold_weights(wfr, wfi, wbr, wbi):
    wfr = np.asarray(wfr, dtype=np.float64)
    wfi = np.asarray(wfi, dtype=np.float64)
    wbr = np.asarray(wbr, dtype=np.float64)
    wbi = np.asarray(wbi, dtype=np.float64)
    A = wfr.T @ wbr - wfi.T @ wbi  # [800, 800]
    Ab = A.reshape(4, H, 4, H)     # [r, k, j, c] blocks
    C = np.zeros((7, H, H))
    for d in range(-3, 4):
        for j in range(4):
            r = j + d
            if 0 <= r <= 3:
                C[d + 3] += Ab[r, :, j, :]
    # cm[k, (d+3)*H + c] = C[d, k, c]
    cm = np.ascontiguousarray(C.transpose(1, 0, 2).reshape(H, 7 * H))
    # edge corrections, NEGATED so the device just accumulates and adds.
    # lo (first out block, g=2):  -= sum_t u[t]      @ A_blk[1+t, 3]
    # hi (last out block, g=2401): -= sum_t u[2401+t] @ A_blk[t, 0]
    E = np.zeros((2, 3, H, H))
    for t in range(3):
        E[0, t] = -Ab[1 + t, :, 3, :]
        E[1, t] = -Ab[t, :, 0, :]
    # ce[k, (e*3+t)*H + c] = E[e, t, k, c]
    ce = np.ascontiguousarray(E.transpose(2, 0, 1, 3).reshape(H, 6 * H))
    return cm.astype(np.float32), ce.astype(np.float32)


# ---------------- bass program ----------------
def _build_nc():
    import concourse.bass as bass
    import concourse.mybir as mybir
    from concourse.tile import TileContext
    from concourse.tile_rust import add_dep_helper

    mmdt = getattr(mybir.dt, _MM_DTYPE)
    f32 = mybir.dt.float32

    nc = bass.Bass()
    xt_d = nc.declare_dram_parameter("xt", [H, BPC * NCOL], mmdt, False)
    cm_d = nc.declare_dram_parameter("cm", [H, 7 * H], mmdt, False)
    ce_d = nc.declare_dram_parameter("ce", [H, 6 * H], mmdt, False)
    eg_d = nc.declare_dram_parameter("eg", [H, 6 * BPC], mmdt, False)
    yt_d = nc.declare_dram_parameter("yt", [H, BPC * G], f32, True)

    with TileContext(nc) as tc:
        with (
            tc.tile_pool(name="wpool", bufs=1) as wpool,
            tc.tile_pool(name="xpool", bufs=1) as xpool,
            tc.tile_pool(name="opool0", bufs=4) as opool0,
            tc.tile_pool(name="opool1", bufs=4) as opool1,
            tc.tile_pool(name="epool", bufs=1) as epool,
            tc.tile_pool(name="pmain", bufs=6, space="PSUM") as pmain,
            tc.tile_pool(name="pedge", bufs=2, space="PSUM") as pedge,
        ):
            opools = (opool0, opool1)
            cm_t, ce_t, xt_t, eg_t = {}, {}, {}, {}
            # critical-path DMAs first: main weights + batch-0 x in
            # per-group chunks so grp0 can start within ~10us
            for kci, (k0, kn) in enumerate(KC):
                cm_t[kci] = wpool.tile([kn, 7 * H], mmdt, name=f"cm{kci}", tag=f"cm{kci}")
                nc.sync.dma_start(out=cm_t[kci][:], in_=cm_d[k0:k0 + kn, :])
                xt_t[kci] = xpool.tile([kn, BPC * NCOL], mmdt, name=f"xt{kci}", tag=f"xt{kci}")
            bounds = [0, 487, 967, 1447, 1927, NCOL]
            for ci in range(5):
                lo, hi = bounds[ci], bounds[ci + 1]
                for kci, (k0, kn) in enumerate(KC):
                    nc.sync.dma_start(
                        out=xt_t[kci][:, lo:hi], in_=xt_d[k0:k0 + kn, lo:hi]
                    )
            for kci, (k0, kn) in enumerate(KC):
                ce_t[kci] = wpool.tile([kn, 6 * H], mmdt, name=f"ce{kci}", tag=f"ce{kci}")
                nc.sync.dma_start(out=ce_t[kci][:], in_=ce_d[k0:k0 + kn, :])
                eg_t[kci] = epool.tile([kn, 6 * BPC], mmdt, name=f"eg{kci}", tag=f"eg{kci}")
                nc.sync.dma_start(out=eg_t[kci][:], in_=eg_d[k0:k0 + kn, :])
            for b in range(1, BPC):
                for kci, (k0, kn) in enumerate(KC):
                    nc.sync.dma_start(
                        out=xt_t[kci][:, b * NCOL:(b + 1) * NCOL],
                        in_=xt_d[k0:k0 + kn, b * NCOL:(b + 1) * NCOL],
                    )

            esb_t = {}
            for cci, (c0, cn) in enumerate(CC):
                esb_t[cci] = epool.tile([cn, 2 * BPC], f32, name=f"esb{cci}", tag=f"esb{cci}")

            def emit_edges(after_mm):
                # edge-correction matmuls -> esb[cci][:, e*BPC + b]
                for cci, (c0, cn) in enumerate(CC):
                    for e in range(2):
                        pe_t = pedge.tile([cn, BPC], f32, name="pe_t", tag="pe_t")
                        idx = 0
                        for t in range(3):
                            for kci, (k0, kn) in enumerate(KC):
                                s = (e * 3 + t)
                                mm = nc.tensor.matmul(
                                    pe_t[:],
                                    ce_t[kci][:, s * H + c0: s * H + c0 + cn],
                                    eg_t[kci][:, s * BPC:(s + 1) * BPC],
                                    start=(idx == 0),
                                    stop=(idx == 5),
                                )
                                if after_mm is not None:
                                    add_dep_helper(mm.ins, after_mm.ins, sync=False,
                                                   reason="edges after b0 mains")
                                idx += 1
                        nc.vector.tensor_copy(
                            out=esb_t[cci][:, e * BPC:(e + 1) * BPC], in_=pe_t[:]
                        )

            def emit_batch_main(b):
                ots = {}
                last_mm = None
                for cci, (c0, cn) in enumerate(CC):
                    ot = opools[cci].tile([cn, G], f32, name=f"ot{cci}", tag=f"ot{cci}")
                    ots[cci] = ot
                    for grp in range(NGRP):
                        o0 = grp * GRP
                        ps = pmain.tile([cn, GRP], f32, name="ps", tag="ps")
                        idx = 0
                        for d in range(-3, 4):
                            for kci, (k0, kn) in enumerate(KC):
                                last_mm = nc.tensor.matmul(
                                    ps[:],
                                    cm_t[kci][:, (d + 3) * H + c0:(d + 3) * H + c0 + cn],
                                    xt_t[kci][:, b * NCOL + o0 + 3 + d:
                                              b * NCOL + o0 + 3 + d + GRP],
                                    start=(idx == 0),
                                    stop=(idx == 13),
                                )
                                idx += 1
                        nc.vector.tensor_copy(out=ot[:, o0:o0 + GRP], in_=ps[:])
                        if 0 < grp < NGRP - 1:
                            # interior groups stream out immediately
                            nc.sync.dma_start(
                                out=yt_d[c0:c0 + cn, b * G + o0:b * G + o0 + GRP],
                                in_=ot[:, o0:o0 + GRP],
                            )
                return ots, last_mm

            def emit_batch_edges_and_out(b, ots):
                # boundary-block corrections, then first/last group out-DMAs
                for cci, (c0, cn) in enumerate(CC):
                    ot = ots[cci]
                    nc.vector.tensor_add(
                        out=ot[:, 0:1], in0=ot[:, 0:1], in1=esb_t[cci][:, b:b + 1]
                    )
                    nc.vector.tensor_add(
                        out=ot[:, G - 1:G], in0=ot[:, G - 1:G],
                        in1=esb_t[cci][:, BPC + b:BPC + b + 1],
                    )
                    for grp in (0, NGRP - 1):
                        o0 = grp * GRP
                        nc.sync.dma_start(
                            out=yt_d[c0:c0 + cn, b * G + o0:b * G + o0 + GRP],
                            in_=ot[:, o0:o0 + GRP],
                        )

            ots0, last0 = emit_batch_main(0)
            emit_edges(last0)
            emit_batch_edges_and_out(0, ots0)
            for b in range(1, BPC):
                ots, _ = emit_batch_main(b)
                emit_batch_edges_and_out(b, ots)
    return nc


def _legalize_waits(nc):
    """walrus fuses at most ONE sync-wait into most instructions (and the
    Tile kernel-tail drain gets one per outstanding proc).  Split extras
    into preceding single-wait NoOps on the same engine."""
    import concourse.mybir as mybir

    for f in nc.m.functions:
        for blk in f.blocks:
            new, changed = [], False
            for inst in blk.instructions:
                si = inst.sync_info
                if si is not None and si.on_wait and len(si.on_wait) > 1:
                    waits = list(si.on_wait)
                    for i, w in enumerate(waits[:-1]):
                        nop = mybir.InstNoOp(
                            name=f"{inst.name}-waitsplit{i}", ins=[], outs=[])
                        nop.engine = inst.engine
                        nop.sync_info = mybir.SyncInfo(on_wait=[w], on_update=[])
                        new.append(nop)
                    inst.sync_info = mybir.SyncInfo(
                        on_wait=[waits[-1]], on_update=list(si.on_update or []))
                    changed = True
                new.append(inst)
            if changed:
                blk.instructions = new


def _get_nc():
    if "nc" not in _CACHE:
        nc = _build_nc()
        _legalize_waits(nc)
        _CACHE["nc"] = nc
    return _CACHE["nc"]


# ---------------- host-side data layout ----------------
def _prep_x(x):
    """x [B, T] f32 -> per-core xt [H, BPC*NCOL] f32, transposed block layout
    with one zero border column per batch on each side; plus per-core edge
    input columns eg [H, 6*BPC] (lo: blocks 0..2, hi: blocks 2401..2403)."""
    xp = np.pad(np.asarray(x, dtype=np.float32), ((0, 0), (PAD, PAD)), mode="edge")
    blocks = xp.reshape(B, NBLK, H)
    xts, egs = [], []
    for c in range(N_CORES):
        cb = blocks[c * BPC:(c + 1) * BPC]          # [BPC, NBLK, H]
        xt = np.zeros((H, BPC, NCOL), dtype=np.float32)
        # xt[k, b, i] = xp[core_b, (i-1)*H + k]
        xt[:, :, 1:NCOL - 1] = cb.transpose(2, 0, 1)
        xts.append(np.ascontiguousarray(xt.reshape(H, BPC * NCOL)))
        eg = np.empty((H, 2, 3, BPC), dtype=np.float32)
        for t in range(3):
            eg[:, 0, t, :] = cb[:, t, :].T           # u[t]
            eg[:, 1, t, :] = cb[:, 2401 + t, :].T    # u[2401+t]
        egs.append(np.ascontiguousarray(eg.reshape(H, 6 * BPC)))
    return xts, egs


def _gather_y(results):
    out = np.empty((B, T), dtype=np.float32)
    for c in range(N_CORES):
        yt = results[c]["yt"].reshape(H, BPC, G)
        out[c * BPC:(c + 1) * BPC] = (
            yt.transpose(1, 2, 0).reshape(BPC, T)
        )
    return out


# ---------------- entry point ----------------
def kernel(x, w_fwd_real, w_fwd_imag, w_bwd_real, w_bwd_imag, **_):
    from concourse.bass_utils import run_bass_kernel_spmd

    cm, ce = _fold_weights(w_fwd_real, w_fwd_imag, w_bwd_real, w_bwd_imag)
    xts, egs = _prep_x(x)
    in_maps = [{"xt": xts[c], "cm": cm, "ce": ce, "eg": egs[c]}
               for c in range(N_CORES)]
    nc = _get_nc()
    res = run_bass_kernel_spmd(nc, in_maps, list(range(N_CORES)))
    return _gather_y(res.results)
